# revision 12
# baseline (speedup 1.0000x reference)
"""NestedAttention Trainium2 kernel (fp8 DoubleRow + multi-engine softmax).

Reference computation (per batch b):
  q_i = wq[i] @ x ; k_j = wk[j] @ x ; v_j = wv[j] @ x        (1x1 convs, r=64)
  for i: acc_i = sum_j softmax_m(q_i^T k_j / sqrt(r)) applied to v_j
  out = wo @ concat_i(acc_i) ; y = x * sigmoid(out)

Sharding: 8 cores = batch(4) x query-column-halves(2). Each core holds full
k/v (m = 2304 keys) and a 1152-wide slice of query columns n; no cross-core
communication (softmax is over m, fully on-core).

Key tricks vs the bf16 baseline:
  * All projection matmuls and mm2 run in fp8e4m3 with MatmulPerfMode.DoubleRow
    (contraction 2x128 per pass), halving PE instruction-column counts.
  * exp() is replaced by a Schraudolph bit-trick: q is pre-scaled by
    0.125*log2(e)*8 so mm1 logits are 8*log2(E); an extra contraction row
    (q_row=B, k_row=1) folds in the exponent-bias constant, so PSUM holds the
    int8 BIT PATTERN of fp8e4m3(E) as an f32 number. E-tiles are then produced
    by a plain f32->int8 convert-copy, which Vector AND GpSimd can run (the
    Scalar engine keeps true exp for its share of tiles, selected per m-tile
    via the k bias-row mask). This splits the 23.9M-elem softmax across 3
    engines instead of serializing on Scalar.
  * Softmax normalization: mm2's stationary is [v^T | ones] so PSUM rows 0:64
    hold the numerator and 64:128 hold Z; one shifted copy + reciprocal +
    multiply per chunk writes acc_ij (bf16). The j-sum is folded into the
    final wo projection's PSUM accumulation (9 terms) instead of DVE adds.
"""

import os
import numpy as np

B, C, H, W = 4, 256, 48, 48
N = H * W            # 2304 keys (m) per image
NSLICE = N // 2      # 1152 query columns (n) per core
R = 64               # reduced channels
P = 128
MT = N // P          # 18 m-tiles
MT2 = MT // 2        # 9 double m-tiles for DoubleRow mm2
CHUNKS = [(0, 512), (512, 512), (1024, 128)]  # n chunks, PSUM-bank aligned
N_CORES = 8

LOG2E8 = 1.4426950408889634          # 0.125 * log2(e) * 8 folded into wq
EXP_SCALE = 0.6931471805599453 / 8.0  # recovers exp(s/8) from 8*log2(E)
BIAS_BITS = 56.25                     # 8*(7 + c) Schraudolph bias constant

# env-tunable experiment knobs (compile-time)
MM1_MODE = os.environ.get("NESTED_MM1", "bf16")       # bf16 | fp8dr
_es = os.environ.get("NESTED_ESPLIT", "10,8")
ACT_T, DVE_T = (int(v) for v in _es.split(","))
assert ACT_T + DVE_T == MT  # gpsimd has no PSUM access -> only Act/DVE make E

_CACHE = {}
LAST_RESULTS = None


def _build_program():
    from contextlib import ExitStack

    import concourse.bass as bass
    import concourse.tile as tile
    from concourse import bacc, mybir

    f32 = mybir.dt.float32
    bf16 = mybir.dt.bfloat16
    fp8 = mybir.dt.float8e4
    i8 = mybir.dt.int8
    Exp = mybir.ActivationFunctionType.Exp
    Sigmoid = mybir.ActivationFunctionType.Sigmoid
    mult = mybir.AluOpType.mult
    DR = mybir.MatmulPerfMode.DoubleRow

    nc = bacc.Bacc("TRN2", target_bir_lowering=False, debug=False)
    # x channels split [2, 128]: x2_d[s, p, n] = x[s*128+p, n]
    x2_d = nc.declare_dram_parameter("x2", [2, P, N], fp8, isOutput=False)
    xn2_d = nc.declare_dram_parameter("xn2", [2, P, NSLICE], fp8, isOutput=False)
    xn_d = nc.declare_dram_parameter("xn", [2, P, NSLICE], f32, isOutput=False)
    wq2_d = nc.declare_dram_parameter("wq2", [P, 2, 3, R], fp8, isOutput=False)
    wk2_d = nc.declare_dram_parameter("wk2", [P, 2, 3, R], fp8, isOutput=False)
    wv2_d = nc.declare_dram_parameter("wv2", [P, 2, 3, R], fp8, isOutput=False)
    woT_d = nc.declare_dram_parameter("woT", [3, R, C], bf16, isOutput=False)
    y_d = nc.declare_dram_parameter("y", [2, P, NSLICE], f32, isOutput=True)

    conv_m0 = ACT_T * P  # m >= conv_m0 uses the Schraudolph convert path

    with tile.TileContext(nc) as tc, ExitStack() as ctx:
        consts = ctx.enter_context(tc.tile_pool(name="consts", bufs=1))
        big_ps = ctx.enter_context(tc.tile_pool(name="big_ps", bufs=2, space="PSUM"))
        mm2_ps = ctx.enter_context(tc.tile_pool(name="mm2_ps", bufs=2, space="PSUM"))
        e_pool = ctx.enter_context(tc.tile_pool(name="e_pool", bufs=2))
        rb_pool = ctx.enter_context(tc.tile_pool(name="rb_pool", bufs=2))
        small = ctx.enter_context(tc.tile_pool(name="small", bufs=2))

        # ---- persistent SBUF state ----
        wq2_sb = consts.tile([P, 2, 3, R], fp8)
        nc.sync.dma_start(wq2_sb[:], wq2_d[:])
        xn2_sb = consts.tile([P, 2, NSLICE], fp8)
        nc.sync.dma_start(xn2_sb[:], xn2_d.rearrange("t p m -> p t m"))
        wk2_sb = consts.tile([P, 2, 3, R], fp8)
        nc.sync.dma_start(wk2_sb[:], wk2_d[:])
        x2_sb = consts.tile([P, 2, N], fp8)
        nc.sync.dma_start(
            x2_sb[:, :, 0:NSLICE], x2_d[:, :, 0:NSLICE].rearrange("t p m -> p t m")
        )
        nc.sync.dma_start(
            x2_sb[:, :, NSLICE:N], x2_d[:, :, NSLICE:N].rearrange("t p m -> p t m")
        )
        wv2_sb = consts.tile([P, 2, 3, R], fp8)
        nc.sync.dma_start(wv2_sb[:], wv2_d[:])
        xn_sb = consts.tile([P, 2, NSLICE], f32)
        nc.sync.dma_start(xn_sb[:], xn_d.rearrange("t p m -> p t m"))

        woT_sb = []
        for i in range(3):
            w = consts.tile([R, C], bf16, tag=f"woT{i}")
            nc.sync.dma_start(w[:], woT_d[i])
            woT_sb.append(w)

        # q/k buffers for mm1 (+ bias row 64 for the Schraudolph constant)
        if MM1_MODE == "bf16":
            q_sb = consts.tile([P, 3, NSLICE], bf16)
            nc.vector.memset(q_sb[R:P, :, :], 0.0)
            nc.gpsimd.memset(q_sb[R : R + 1, :, :], BIAS_BITS)
            k_sb = consts.tile([P, 3, N], bf16)
            nc.vector.memset(k_sb[R:P, :, :], 0.0)
            if conv_m0 > 0:
                nc.gpsimd.memset(k_sb[R : R + 1, :, 0:conv_m0], 0.0)
            if conv_m0 < N:
                nc.gpsimd.memset(k_sb[R : R + 1, :, conv_m0:N], 1.0)
        else:
            # fp8 DoubleRow mm1: [65, 2, ...] with sub1 zero, row 64 = bias
            q_sb = consts.tile([P, 2, 3, NSLICE], fp8)
            nc.vector.memset(q_sb[:, 1, :, :], 0.0)
            nc.vector.memset(q_sb[R:P, 0, :, :], 0.0)
            nc.gpsimd.memset(q_sb[R : R + 1, 0, :, :], 56.0)
            k_sb = consts.tile([P, 2, 3, N], fp8)
            nc.vector.memset(k_sb[:, 1, :, :], 0.0)
            nc.vector.memset(k_sb[R:P, 0, :, :], 0.0)
            if conv_m0 > 0:
                nc.gpsimd.memset(k_sb[R : R + 1, 0, :, 0:conv_m0], 0.0)
            if conv_m0 < N:
                nc.gpsimd.memset(k_sb[R : R + 1, 0, :, conv_m0:N], 1.0)

        # vT buffer per double-m-tile: [(vT_0|ones)(vT_1|ones)(vT_2|ones)] x2 sub
        vT_buf = consts.tile([P, MT2, 2, 384], fp8)
        for j in range(3):
            nc.gpsimd.memset(vT_buf[:, :, :, 128 * j + 64 : 128 * j + 128], 1.0)

        # acc_ij in bf16; j-sum folded into final projection's PSUM accum
        acc = {}
        for i in range(3):
            for j in range(3):
                acc[(i, j)] = consts.tile(
                    [R, NSLICE], bf16, tag=f"acc{i}{j}", name=f"acc{i}{j}"
                )

        # ---- projections (fp8 DoubleRow, K=256 contracted in one pass) ----
        def emit_q(i):
            pt = big_ps.tile([P, NSLICE], f32, tag="big")
            for c0, cw in CHUNKS:
                nc.tensor.matmul(
                    pt[0:R, c0 : c0 + cw],
                    wq2_sb[:, :, i, :],
                    xn2_sb[:, :, c0 : c0 + cw],
                    start=True,
                    stop=True,
                    perf_mode=DR,
                )
            if MM1_MODE == "bf16":
                nc.vector.tensor_copy(q_sb[0:R, i, :], pt[0:R, :])
            else:
                nc.vector.tensor_copy(q_sb[0:R, 0, i, :], pt[0:R, :])

        def emit_k(j, halves=(0, 1)):
            for half in halves:
                pt = big_ps.tile([P, NSLICE], f32, tag="big")
                for c0, cw in CHUNKS:
                    nc.tensor.matmul(
                        pt[0:R, c0 : c0 + cw],
                        wk2_sb[:, :, j, :],
                        x2_sb[:, :, half * NSLICE + c0 : half * NSLICE + c0 + cw],
                        start=True,
                        stop=True,
                        perf_mode=DR,
                    )
                if MM1_MODE == "bf16":
                    nc.scalar.copy(
                        k_sb[0:R, j, half * NSLICE : (half + 1) * NSLICE], pt[0:R, :]
                    )
                else:
                    nc.scalar.copy(
                        k_sb[0:R, 0, j, half * NSLICE : (half + 1) * NSLICE],
                        pt[0:R, :],
                    )

        def emit_vT_all():
            for mt in range(MT):
                pv = mm2_ps.tile([P, 512], f32, tag="mm2")
                nc.tensor.matmul(
                    pv[:, 0 : 3 * R],
                    x2_sb[:, :, mt * P : (mt + 1) * P],
                    wv2_sb[:, :, :, :],
                    start=True,
                    stop=True,
                    perf_mode=DR,
                )
                base = vT_buf[:, mt // 2, mt % 2, :]
                dst = bass.AP(
                    tensor=base.tensor,
                    offset=base.offset,
                    ap=[base.ap[0], [128, 3], [1, R]],
                )
                nc.scalar.copy(
                    dst, pv[:, 0 : 3 * R].rearrange("p (j r) -> p j r", j=3)
                )

        # ---- attention pair pipeline ----
        def emit_mm1_exp(i, j):
            E = e_pool.tile([P, MT2, 2, NSLICE], fp8, tag="E")
            for mt in range(MT):
                pt = big_ps.tile([P, NSLICE], f32, tag="big")
                for c0, cw in CHUNKS:
                    if MM1_MODE == "bf16":
                        nc.tensor.matmul(
                            pt[:, c0 : c0 + cw],
                            k_sb[:, j, mt * P : (mt + 1) * P],
                            q_sb[:, i, c0 : c0 + cw],
                            start=True,
                            stop=True,
                        )
                    else:
                        nc.tensor.matmul(
                            pt[:, c0 : c0 + cw],
                            k_sb[0 : R + 1, :, j, mt * P : (mt + 1) * P],
                            q_sb[0 : R + 1, :, i, c0 : c0 + cw],
                            start=True,
                            stop=True,
                            perf_mode=DR,
                        )
                dst = E[:, mt // 2, mt % 2, :]
                if mt < ACT_T:
                    nc.scalar.activation(dst, pt[:], Exp, scale=EXP_SCALE)
                else:
                    nc.vector.tensor_copy(dst.bitcast(i8), pt[:])
            return E

        po = [None, None]

        def emit_final_chunk(c0, cw):
            for mtile in range(2):
                for i in range(3):
                    for j in range(3):
                        nc.tensor.matmul(
                            po[mtile][:, c0 : c0 + cw],
                            woT_sb[i][:, mtile * P : (mtile + 1) * P],
                            acc[(i, j)][:, c0 : c0 + cw],
                            start=(i == 0 and j == 0),
                            stop=(i == 2 and j == 2),
                        )
            for mtile in range(2):
                sig = small.tile([P, 512], f32, tag="sig")
                nc.scalar.activation(sig[:, 0:cw], po[mtile][:, c0 : c0 + cw], Sigmoid)
                y_sb = small.tile([P, 512], f32, tag="ysb")
                nc.gpsimd.tensor_tensor(
                    y_sb[:, 0:cw], xn_sb[:, mtile, c0 : c0 + cw], sig[:, 0:cw], mult
                )
                nc.sync.dma_start(y_d[mtile][:, c0 : c0 + cw], y_sb[:, 0:cw])

        def emit_mm2_norm(i, j, E, last=False):
            for c0, cw in CHUNKS:
                pa = mm2_ps.tile([P, 512], f32, tag="mm2")
                for t in range(MT2):
                    nc.tensor.matmul(
                        pa[:, 0:cw],
                        vT_buf[:, t, :, 128 * j : 128 * (j + 1)],
                        E[:, t, :, c0 : c0 + cw],
                        start=(t == 0),
                        stop=(t == MT2 - 1),
                        perf_mode=DR,
                    )
                rb = rb_pool.tile([R, 512], f32, tag="rb")
                nc.vector.tensor_copy(rb[:, 0:cw], pa[R:P, 0:cw])
                nc.vector.reciprocal_approx_fast(rb[:, 0:cw], rb[:, 0:cw])
                nc.vector.tensor_tensor(
                    acc[(i, j)][:, c0 : c0 + cw], pa[0:R, 0:cw], rb[:, 0:cw], mult
                )
                if last:
                    emit_final_chunk(c0, cw)

        pairs = [(i, j) for j in range(3) for i in range(3)]
        prev = None
        for idx, (i, j) in enumerate(pairs):
            if idx == 0:
                emit_q(0)
                emit_k(0)
            E = emit_mm1_exp(i, j)
            if idx == 0:
                emit_q(1)
                emit_q(2)
            elif idx == 1:
                emit_vT_all()
            elif idx == 2:
                emit_k(1)
            elif idx == 4:
                emit_k(2)
            if prev is not None:
                emit_mm2_norm(prev[0], prev[1], prev[2])
            prev = (i, j, E)
        po[0] = big_ps.tile([P, NSLICE], f32, tag="big", name="po0")
        po[1] = big_ps.tile([P, NSLICE], f32, tag="big", name="po1")
        emit_mm2_norm(prev[0], prev[1], prev[2], last=True)

    nc.compile()
    return nc


def _get_program():
    if "nc" not in _CACHE:
        _CACHE["nc"] = _build_program()
    return _CACHE["nc"]


def _host_prep(x, wq, wk, wv, wo):
    import ml_dtypes

    bf16 = ml_dtypes.bfloat16
    fp8 = ml_dtypes.float8_e4m3fn
    xf = np.ascontiguousarray(x.reshape(B, C, N), dtype=np.float32)
    # wq scaled so mm1 logits are 8*log2(E); layout [p, s, i, r] = w[i, r, s*128+p]
    def wlayout(w, scale=1.0):
        wt = (np.asarray(w, np.float32) * scale).transpose(2, 0, 1)  # [C, 3, R]
        return np.ascontiguousarray(
            wt.reshape(2, P, 3, R).transpose(1, 0, 2, 3)
        ).astype(fp8)

    wq2 = wlayout(wq, LOG2E8)
    wk2 = wlayout(wk)
    wv2 = wlayout(wv)
    # wo: [C, 3R] -> woT[i] = wo[:, 64i:64(i+1)].T
    woT = np.ascontiguousarray(
        np.stack([np.asarray(wo, np.float32)[:, R * i : R * (i + 1)].T for i in range(3)])
    ).astype(bf16)
    in_maps = []
    for core in range(N_CORES):
        b, h = core // 2, core % 2
        xcore = xf[b].reshape(2, P, N)
        xn32 = np.ascontiguousarray(xcore[:, :, h * NSLICE : (h + 1) * NSLICE])
        in_maps.append(
            {
                "x2": xcore.astype(fp8),
                "xn2": xn32.astype(fp8),
                "xn": xn32,
                "wq2": wq2,
                "wk2": wk2,
                "wv2": wv2,
                "woT": woT,
            }
        )
    return in_maps


def kernel(x, wq, wk, wv, wo):
    global LAST_RESULTS
    from concourse.bass_utils import run_bass_kernel_spmd

    x = np.asarray(x)
    nc = _get_program()
    in_maps = _host_prep(
        x, np.asarray(wq), np.asarray(wk), np.asarray(wv), np.asarray(wo)
    )
    res = run_bass_kernel_spmd(nc, in_maps, core_ids=list(range(N_CORES)))
    LAST_RESULTS = res
    out = np.empty((B, C, N), np.float32)
    for core in range(N_CORES):
        b, h = core // 2, core % 2
        out[b][:, h * NSLICE : (h + 1) * NSLICE] = res.results[core]["y"].reshape(
            C, NSLICE
        )
    return out.reshape(B, C, H, W).astype(x.dtype, copy=False)


# revision 18
# speedup vs baseline: 1.0931x; 1.0931x over previous
"""NestedAttention Trainium2 kernel (fp8 DoubleRow + multi-engine softmax).

Reference computation (per batch b):
  q_i = wq[i] @ x ; k_j = wk[j] @ x ; v_j = wv[j] @ x        (1x1 convs, r=64)
  for i: acc_i = sum_j softmax_m(q_i^T k_j / sqrt(r)) applied to v_j
  out = wo @ concat_i(acc_i) ; y = x * sigmoid(out)

Sharding: 8 cores = batch(4) x query-column-halves(2). Each core holds full
k/v (m = 2304 keys) and a 1152-wide slice of query columns n; no cross-core
communication (softmax is over m, fully on-core).

Key tricks vs the bf16 baseline:
  * All projection matmuls and mm2 run in fp8e4m3 with MatmulPerfMode.DoubleRow
    (contraction 2x128 per pass), halving PE instruction-column counts.
  * exp() is replaced by a Schraudolph bit-trick: q is pre-scaled by
    0.125*log2(e)*8 so mm1 logits are 8*log2(E); an extra contraction row
    (q_row=B, k_row=1) folds in the exponent-bias constant, so PSUM holds the
    int8 BIT PATTERN of fp8e4m3(E) as an f32 number. E-tiles are then produced
    by a plain f32->int8 convert-copy, which Vector AND GpSimd can run (the
    Scalar engine keeps true exp for its share of tiles, selected per m-tile
    via the k bias-row mask). This splits the 23.9M-elem softmax across 3
    engines instead of serializing on Scalar.
  * Softmax normalization: mm2's stationary is [v^T | ones] so PSUM rows 0:64
    hold the numerator and 64:128 hold Z; one shifted copy + reciprocal +
    multiply per chunk writes acc_ij (bf16). The j-sum is folded into the
    final wo projection's PSUM accumulation (9 terms) instead of DVE adds.
"""

import os
import numpy as np

B, C, H, W = 4, 256, 48, 48
N = H * W            # 2304 keys (m) per image
NSLICE = N // 2      # 1152 query columns (n) per core
R = 64               # reduced channels
P = 128
MT = N // P          # 18 m-tiles
MT2 = MT // 2        # 9 double m-tiles for DoubleRow mm2
CHUNKS = [(0, 512), (512, 512), (1024, 128)]  # n chunks, PSUM-bank aligned
N_CORES = 8

LOG2E8 = 1.4426950408889634          # 0.125 * log2(e) * 8 folded into wq
EXP_SCALE = 0.6931471805599453 / 8.0  # recovers exp(s/8) from 8*log2(E)
BIAS_BITS = 56.25                     # 8*(7 + c) Schraudolph bias constant

# env-tunable experiment knobs (compile-time)
MM1_MODE = os.environ.get("NESTED_MM1", "bf16")       # bf16 | fp8dr
_es = os.environ.get("NESTED_ESPLIT", "12,6")
ACT_T, DVE_T = (int(v) for v in _es.split(","))
assert ACT_T + DVE_T == MT  # gpsimd has no PSUM access -> only Act/DVE make E
# interleave DVE-assigned m-tiles among Act's so both engines run concurrently
DVE_SET = {int((k + 0.5) * MT / DVE_T) for k in range(DVE_T)} if DVE_T else set()

_CACHE = {}
LAST_RESULTS = None


def _build_program():
    from contextlib import ExitStack

    import concourse.bass as bass
    import concourse.tile as tile
    from concourse import bacc, mybir

    f32 = mybir.dt.float32
    bf16 = mybir.dt.bfloat16
    fp8 = mybir.dt.float8e4
    i8 = mybir.dt.int8
    Exp = mybir.ActivationFunctionType.Exp
    Sigmoid = mybir.ActivationFunctionType.Sigmoid
    mult = mybir.AluOpType.mult
    DR = mybir.MatmulPerfMode.DoubleRow

    nc = bacc.Bacc("TRN2", target_bir_lowering=False, debug=False)
    assert len(DVE_SET) == DVE_T
    # x channels split [2, 128]: x2_d[s, p, n] = x[s*128+p, n]
    x2_d = nc.declare_dram_parameter("x2", [2, P, N], fp8, isOutput=False)
    xn2_d = nc.declare_dram_parameter("xn2", [2, P, NSLICE], fp8, isOutput=False)
    xn_d = nc.declare_dram_parameter("xn", [2, P, NSLICE], f32, isOutput=False)
    wq2_d = nc.declare_dram_parameter("wq2", [P, 2, 3, R], fp8, isOutput=False)
    wk2_d = nc.declare_dram_parameter("wk2", [P, 2, 3, R], fp8, isOutput=False)
    wv2_d = nc.declare_dram_parameter("wv2", [P, 2, 3, R], fp8, isOutput=False)
    woT_d = nc.declare_dram_parameter("woT", [3, R, C], bf16, isOutput=False)
    y_d = nc.declare_dram_parameter("y", [2, P, NSLICE], f32, isOutput=True)



    with tile.TileContext(nc) as tc, ExitStack() as ctx:
        consts = ctx.enter_context(tc.tile_pool(name="consts", bufs=1))
        big_ps = ctx.enter_context(tc.tile_pool(name="big_ps", bufs=2, space="PSUM"))
        mm2_ps = ctx.enter_context(tc.tile_pool(name="mm2_ps", bufs=2, space="PSUM"))
        e_pool = ctx.enter_context(tc.tile_pool(name="e_pool", bufs=2))
        rb_pool = ctx.enter_context(tc.tile_pool(name="rb_pool", bufs=2))
        small = ctx.enter_context(tc.tile_pool(name="small", bufs=2))

        # ---- persistent SBUF state ----
        wq2_sb = consts.tile([P, 2, 3, R], fp8)
        nc.sync.dma_start(wq2_sb[:], wq2_d[:])
        xn2_sb = consts.tile([P, 2, NSLICE], fp8)
        nc.sync.dma_start(xn2_sb[:], xn2_d.rearrange("t p m -> p t m"))
        wk2_sb = consts.tile([P, 2, 3, R], fp8)
        nc.sync.dma_start(wk2_sb[:], wk2_d[:])
        x2_sb = consts.tile([P, 2, N], fp8)
        nc.sync.dma_start(
            x2_sb[:, :, 0:NSLICE], x2_d[:, :, 0:NSLICE].rearrange("t p m -> p t m")
        )
        nc.sync.dma_start(
            x2_sb[:, :, NSLICE:N], x2_d[:, :, NSLICE:N].rearrange("t p m -> p t m")
        )
        wv2_sb = consts.tile([P, 2, 3, R], fp8)
        nc.sync.dma_start(wv2_sb[:], wv2_d[:])
        xn_sb = consts.tile([P, 2, NSLICE], f32)
        nc.sync.dma_start(xn_sb[:], xn_d.rearrange("t p m -> p t m"))

        woT_sb = []
        for i in range(3):
            w = consts.tile([R, C], bf16, tag=f"woT{i}")
            nc.sync.dma_start(w[:], woT_d[i])
            woT_sb.append(w)

        # q/k buffers for mm1 (+ bias row 64 for the Schraudolph constant)
        if MM1_MODE == "bf16":
            q_sb = consts.tile([P, 3, NSLICE], bf16)
            nc.vector.memset(q_sb[R:P, :, :], 0.0)
            nc.gpsimd.memset(q_sb[R : R + 1, :, :], BIAS_BITS)
            k_sb = consts.tile([P, 3, N], bf16)
            nc.vector.memset(k_sb[R:P, :, :], 0.0)
            for mt in sorted(DVE_SET):
                nc.gpsimd.memset(k_sb[R : R + 1, :, mt * P : (mt + 1) * P], 1.0)
        else:
            # fp8 DoubleRow mm1: [65, 2, ...] with sub1 zero, row 64 = bias
            q_sb = consts.tile([P, 2, 3, NSLICE], fp8)
            nc.vector.memset(q_sb[:, 1, :, :], 0.0)
            nc.vector.memset(q_sb[R:P, 0, :, :], 0.0)
            nc.gpsimd.memset(q_sb[R : R + 1, 0, :, :], 56.0)
            k_sb = consts.tile([P, 2, 3, N], fp8)
            nc.vector.memset(k_sb[:, 1, :, :], 0.0)
            nc.vector.memset(k_sb[R:P, 0, :, :], 0.0)
            for mt in sorted(DVE_SET):
                nc.gpsimd.memset(k_sb[R : R + 1, 0, :, mt * P : (mt + 1) * P], 1.0)

        # vT buffer per double-m-tile: [(vT_0|ones)(vT_1|ones)(vT_2|ones)] x2 sub
        vT_buf = consts.tile([P, MT2, 2, 384], fp8)
        for j in range(3):
            nc.gpsimd.memset(vT_buf[:, :, :, 128 * j + 64 : 128 * j + 128], 1.0)

        # acc_ij in bf16; j-sum folded into final projection's PSUM accum
        acc = {}
        for i in range(3):
            for j in range(3):
                acc[(i, j)] = consts.tile(
                    [R, NSLICE], bf16, tag=f"acc{i}{j}", name=f"acc{i}{j}"
                )

        # ---- projections (fp8 DoubleRow, K=256 contracted in one pass) ----
        def emit_q(i):
            pt = big_ps.tile([P, NSLICE], f32, tag="big")
            for c0, cw in CHUNKS:
                nc.tensor.matmul(
                    pt[0:R, c0 : c0 + cw],
                    wq2_sb[:, :, i, :],
                    xn2_sb[:, :, c0 : c0 + cw],
                    start=True,
                    stop=True,
                    perf_mode=DR,
                )
            if MM1_MODE == "bf16":
                nc.vector.tensor_copy(q_sb[0:R, i, :], pt[0:R, :])
            else:
                nc.vector.tensor_copy(q_sb[0:R, 0, i, :], pt[0:R, :])

        def emit_k(j, halves=(0, 1)):
            for half in halves:
                pt = big_ps.tile([P, NSLICE], f32, tag="big")
                for c0, cw in CHUNKS:
                    nc.tensor.matmul(
                        pt[0:R, c0 : c0 + cw],
                        wk2_sb[:, :, j, :],
                        x2_sb[:, :, half * NSLICE + c0 : half * NSLICE + c0 + cw],
                        start=True,
                        stop=True,
                        perf_mode=DR,
                    )
                if MM1_MODE == "bf16":
                    nc.scalar.copy(
                        k_sb[0:R, j, half * NSLICE : (half + 1) * NSLICE], pt[0:R, :]
                    )
                else:
                    nc.scalar.copy(
                        k_sb[0:R, 0, j, half * NSLICE : (half + 1) * NSLICE],
                        pt[0:R, :],
                    )

        def emit_vT_all():
            for mt in range(MT):
                pv = mm2_ps.tile([P, 512], f32, tag="mm2")
                nc.tensor.matmul(
                    pv[:, 0 : 3 * R],
                    x2_sb[:, :, mt * P : (mt + 1) * P],
                    wv2_sb[:, :, :, :],
                    start=True,
                    stop=True,
                    perf_mode=DR,
                )
                base = vT_buf[:, mt // 2, mt % 2, :]
                dst = bass.AP(
                    tensor=base.tensor,
                    offset=base.offset,
                    ap=[base.ap[0], [128, 3], [1, R]],
                )
                nc.scalar.copy(
                    dst, pv[:, 0 : 3 * R].rearrange("p (j r) -> p j r", j=3)
                )

        # ---- attention pair pipeline ----
        def emit_mm1_exp(i, j):
            E = e_pool.tile([P, MT2, 2, NSLICE], fp8, tag="E")
            for mt in range(MT):
                pt = big_ps.tile([P, NSLICE], f32, tag="big")
                for c0, cw in CHUNKS:
                    if MM1_MODE == "bf16":
                        nc.tensor.matmul(
                            pt[:, c0 : c0 + cw],
                            k_sb[:, j, mt * P : (mt + 1) * P],
                            q_sb[:, i, c0 : c0 + cw],
                            start=True,
                            stop=True,
                        )
                    else:
                        nc.tensor.matmul(
                            pt[:, c0 : c0 + cw],
                            k_sb[0 : R + 1, :, j, mt * P : (mt + 1) * P],
                            q_sb[0 : R + 1, :, i, c0 : c0 + cw],
                            start=True,
                            stop=True,
                            perf_mode=DR,
                        )
                dst = E[:, mt // 2, mt % 2, :]
                if mt in DVE_SET:
                    nc.vector.tensor_copy(dst.bitcast(i8), pt[:])
                else:
                    nc.scalar.activation(dst, pt[:], Exp, scale=EXP_SCALE)
            return E

        po = [None, None]

        def emit_final_chunk(c0, cw):
            for mtile in range(2):
                for i in range(3):
                    for j in range(3):
                        nc.tensor.matmul(
                            po[mtile][:, c0 : c0 + cw],
                            woT_sb[i][:, mtile * P : (mtile + 1) * P],
                            acc[(i, j)][:, c0 : c0 + cw],
                            start=(i == 0 and j == 0),
                            stop=(i == 2 and j == 2),
                        )
            for mtile in range(2):
                sig = small.tile([P, 512], f32, tag="sig")
                nc.scalar.activation(sig[:, 0:cw], po[mtile][:, c0 : c0 + cw], Sigmoid)
                y_sb = small.tile([P, 512], f32, tag="ysb")
                nc.gpsimd.tensor_tensor(
                    y_sb[:, 0:cw], xn_sb[:, mtile, c0 : c0 + cw], sig[:, 0:cw], mult
                )
                nc.sync.dma_start(y_d[mtile][:, c0 : c0 + cw], y_sb[:, 0:cw])

        def emit_mm2_norm(i, j, E, last=False):
            for c0, cw in CHUNKS:
                pa = mm2_ps.tile([P, 512], f32, tag="mm2")
                for t in range(MT2):
                    nc.tensor.matmul(
                        pa[:, 0:cw],
                        vT_buf[:, t, :, 128 * j : 128 * (j + 1)],
                        E[:, t, :, c0 : c0 + cw],
                        start=(t == 0),
                        stop=(t == MT2 - 1),
                        perf_mode=DR,
                    )
                rb = rb_pool.tile([R, 512], f32, tag="rb")
                nc.vector.tensor_copy(rb[:, 0:cw], pa[R:P, 0:cw])
                nc.vector.reciprocal_approx_fast(rb[:, 0:cw], rb[:, 0:cw])
                nc.vector.tensor_tensor(
                    acc[(i, j)][:, c0 : c0 + cw], pa[0:R, 0:cw], rb[:, 0:cw], mult
                )
                if last:
                    emit_final_chunk(c0, cw)

        pairs = [(i, j) for j in range(3) for i in range(3)]
        prev = None
        for idx, (i, j) in enumerate(pairs):
            if idx == 0:
                emit_q(0)
                emit_k(0)
            E = emit_mm1_exp(i, j)
            if idx == 0:
                emit_q(1)
                emit_q(2)
            elif idx == 1:
                emit_vT_all()
            elif idx == 2:
                emit_k(1)
            elif idx == 4:
                emit_k(2)
            if prev is not None:
                emit_mm2_norm(prev[0], prev[1], prev[2])
            prev = (i, j, E)
        po[0] = big_ps.tile([P, NSLICE], f32, tag="big", name="po0")
        po[1] = big_ps.tile([P, NSLICE], f32, tag="big", name="po1")
        emit_mm2_norm(prev[0], prev[1], prev[2], last=True)

    nc.compile()
    return nc


def _get_program():
    if "nc" not in _CACHE:
        _CACHE["nc"] = _build_program()
    return _CACHE["nc"]


def _host_prep(x, wq, wk, wv, wo):
    import ml_dtypes

    bf16 = ml_dtypes.bfloat16
    fp8 = ml_dtypes.float8_e4m3fn
    xf = np.ascontiguousarray(x.reshape(B, C, N), dtype=np.float32)
    # wq scaled so mm1 logits are 8*log2(E); layout [p, s, i, r] = w[i, r, s*128+p]
    def wlayout(w, scale=1.0):
        wt = (np.asarray(w, np.float32) * scale).transpose(2, 0, 1)  # [C, 3, R]
        return np.ascontiguousarray(
            wt.reshape(2, P, 3, R).transpose(1, 0, 2, 3)
        ).astype(fp8)

    wq2 = wlayout(wq, LOG2E8)
    wk2 = wlayout(wk)
    wv2 = wlayout(wv)
    # wo: [C, 3R] -> woT[i] = wo[:, 64i:64(i+1)].T
    woT = np.ascontiguousarray(
        np.stack([np.asarray(wo, np.float32)[:, R * i : R * (i + 1)].T for i in range(3)])
    ).astype(bf16)
    in_maps = []
    for core in range(N_CORES):
        b, h = core // 2, core % 2
        xcore = xf[b].reshape(2, P, N)
        xn32 = np.ascontiguousarray(xcore[:, :, h * NSLICE : (h + 1) * NSLICE])
        in_maps.append(
            {
                "x2": xcore.astype(fp8),
                "xn2": xn32.astype(fp8),
                "xn": xn32,
                "wq2": wq2,
                "wk2": wk2,
                "wv2": wv2,
                "woT": woT,
            }
        )
    return in_maps


def kernel(x, wq, wk, wv, wo):
    global LAST_RESULTS
    from concourse.bass_utils import run_bass_kernel_spmd

    x = np.asarray(x)
    nc = _get_program()
    in_maps = _host_prep(
        x, np.asarray(wq), np.asarray(wk), np.asarray(wv), np.asarray(wo)
    )
    res = run_bass_kernel_spmd(nc, in_maps, core_ids=list(range(N_CORES)))
    LAST_RESULTS = res
    out = np.empty((B, C, N), np.float32)
    for core in range(N_CORES):
        b, h = core // 2, core % 2
        out[b][:, h * NSLICE : (h + 1) * NSLICE] = res.results[core]["y"].reshape(
            C, NSLICE
        )
    return out.reshape(B, C, H, W).astype(x.dtype, copy=False)
